# revision 54
# baseline (speedup 1.0000x reference)
"""Trainium2 Bass kernel for a 6-layer transformer encoder.

Problem: B=4, S=512, D=1024, H=16 heads (depth 64), F=4096, L=6 layers, fp32.

Sharding: sequence-sharding within core pairs. Core c handles batch b=c//2,
token half r=c%2. Each core computes Q/attention/Wo/FFN/LN for its own 256
tokens, but K and V for ALL 512 tokens: the own half directly from its own
hidden state (no communication), the peer half from hrem = AllReduce_sum(h)
- h_own. The AllReduce (bf16, 0.5MB) is triggered the moment LN2 finishes a
layer, and the first instruction that depends on it sits behind ~30us of
guaranteed-local PE work (K_own/V_own/Q/QK_local), so the PE never idles
waiting for the collective. Keys are kept in rank-relative order (own 256
first, peer 256 second) so the SPMD program is rank-uniform; softmax is
permutation-invariant over keys so this is exact. The mask and x are packed
host-side in the same rank-relative order.

Precision: weights are pre-cast to bf16 host-side; matmul operands are bf16,
the residual stream (h, r1, h1, y2acc) stays f32r. Softmax uses exp without
max-subtraction (logits are O(1)), row sums via a ones-column appended to V.

Scheduling notes:
- The PE p-state ramps (0.65 -> 1.2 -> 2.4 GHz after 3us continuous busy);
  every idle gap resets it, so phases are ordered to keep one long stream.
- Weights stream through a single 6-buffer ring fed on the sync queue in
  exact consumption order (Q, Wo, W1, W2); Wk/Wv chunks are prefetched one
  layer ahead into held pools during the previous layer's attention, so no
  phase ever waits on a weight chunk.
- Queue discipline: sync = bulk weights only (must never block); scalar
  (hwdge) = latency-critical staging DMAs, each trigger directly after its
  producing activation so it never stalls the queue; gpsimd (swdge, slow) =
  slack-tolerant transfers (oT/sums, hsum unpack, pack+collective). The
  blocking hsum unpack is placed after QK-local's staging on the in-order
  gpsimd queue.
- The AllReduce is split into two pipelined 0.25MB halves, the first
  triggered mid-LN2-apply; LN2's f32r tensor_scalar writes are deferred
  (vector engine) until after the collective trigger.
- Softmax normalize is lag-1 per d-tile (reciprocal_approx_fast every other
  tile); QK-local exps are issued before any AV so exp latency hides behind
  further matmuls.
- PSUM: matmul outputs paired two-per-bank ([*,512] tiles); never mix
  operand base partitions within one bank (matmul operands must sit at base
  partition 0 - base-64 lhsT/rhs faults), never interleave two open
  accumulation groups in one bank. PSUM budget: psA 3 + po 2 + aux 1 +
  stats 2 = 8 banks.
"""

import numpy as np

TO = 256        # own tokens per core
S = 512         # total keys per batch element
D = 1024        # model dim
KD = D // 128   # 8 d-tiles
H = 16          # heads
DH = 64         # head dim
F = 4096        # ff dim
FT = F // 128   # 32 f-tiles
FH = FT // 2    # f-tiles per FFN half
L = 6           # layers
EPS = 1e-6
MAX_POS = 1000
NCORES = 8
WBF = True      # bf16 weights + bf16 matmul operands

_cache = {}


def _imports():
    import sys
    try:
        import concourse.bass  # noqa
    except ImportError:
        for p in ("/opt/trn_rl_repo", "/root/.axon_site/_ro/trn_rl_repo"):
            if p not in sys.path:
                sys.path.insert(0, p)
    import concourse.bass as bass
    import concourse.mybir as mybir
    import concourse.tile as tile
    from concourse import bacc
    from concourse.bass_utils import run_bass_kernel_spmd
    return bass, mybir, tile, bacc, run_bass_kernel_spmd


def build(nlayers=L, use_cc=True, debug=False):
    bass, mybir, tile, bacc, _ = _imports()
    f32 = mybir.dt.float32
    f32r = mybir.dt.float32r
    bf16 = mybir.dt.bfloat16
    AF = mybir.ActivationFunctionType
    OP = mybir.AluOpType
    RG = [[0, 1], [2, 3], [4, 5], [6, 7]]

    nc = bacc.Bacc(None, target_bir_lowering=False, debug=True, num_devices=8)

    # ---- kernel I/O ----
    wdt = bf16 if WBF else f32r
    xTf = nc.declare_dram_parameter("xTf", [D, TO], f32r, isOutput=False)
    xTb = nc.declare_dram_parameter("xTb", [D, S], bf16, isOutput=False)
    msk = nc.declare_dram_parameter("msk", [128, 4], f32, isOutput=False)
    # lhsT-packed: [l, t, kp, k, col] = W[l, k*128+kp, t*128+col]
    WqT = nc.declare_dram_parameter("WqT", [L, KD // 2, 128, 2, KD, 128], wdt,
                                    isOutput=False)
    WkT = nc.declare_dram_parameter("WkT", [L, KD // 2, 128, 2, KD, 128], wdt,
                                    isOutput=False)
    WoT = nc.declare_dram_parameter("WoT", [L, KD // 2, 128, 2, KD, 128], wdt,
                                    isOutput=False)
    # rhs-packed V: [l, nq, kp, k, col] = Wv[l, k*128+kp, nq*256+col]
    WvN = nc.declare_dram_parameter("WvN", [L, 4, 128, KD, 256], wdt, isOutput=False)
    # [l, c, kp, j, k, col] = W1[l, k*128+kp, (2c+j)*128+col]
    W1T = nc.declare_dram_parameter("W1T", [L, FT // 2, 128, 2, KD, 128], wdt,
                                    isOutput=False)
    # [l, ph, m, fp, fo, col] = W2[l, ph*2048 + fo*128+fp, m*128+col]
    W2T = nc.declare_dram_parameter("W2T", [L, 2, KD, 128, FH, 128], wdt,
                                    isOutput=False)
    bias9 = nc.declare_dram_parameter("bias9", [L, 128, KD, 9], f32, isOutput=False)
    b1h = nc.declare_dram_parameter("b1h", [L, 128, FT, 1], f32, isOutput=False)
    cst = nc.declare_dram_parameter("cst", [128, 65], f32r, isOutput=False)   # ones
    cstb = nc.declare_dram_parameter("cstb", [128, 64], bf16, isOutput=False)  # ones
    crow = nc.declare_dram_parameter("crow", [65, 256], f32r, isOutput=False)  # ones
    selc = nc.declare_dram_parameter("selc", [16, KD * 128], f32r, isOutput=False)
    out = nc.declare_dram_parameter("out", [D, TO], f32, isOutput=True)

    with tile.TileContext(nc) as tc:
        with tc.tile_pool(name="sb", bufs=1) as sb1, \
             tc.tile_pool(name="sb2", bufs=2) as sb2, \
             tc.tile_pool(name="wr", bufs=6) as wrp, \
             tc.tile_pool(name="wkp", bufs=4) as wkp, \
             tc.tile_pool(name="wvp", bufs=4) as wvp, \
             tc.tile_pool(name="eap", bufs=16) as eap, \
             tc.tile_pool(name="dram", bufs=2, space="DRAM") as dram, \
             tc.tile_pool(name="psA", bufs=3, space="PSUM") as psA, \
             tc.tile_pool(name="psB", bufs=2, space="PSUM") as psB:

            # ---- persistent tiles ----
            h = sb1.tile([128, KD, TO], f32r, tag="h")
            h_bf = sb1.tile([128, KD, TO], bf16, tag="h_bf")
            hrem = sb1.tile([128, KD, TO], bf16, tag="hrem")
            cst_sb = sb1.tile([128, 65], f32r, tag="cst")
            crow_sb = sb1.tile([65, 256], f32r, tag="crow")
            msk_sb = sb1.tile([128, 4], f32, tag="msk")
            selc_sb = sb1.tile([16, KD * 128], f32r, tag="selc")
            # K for attention: [depth 64, head-in-pair 2, d-tile 8, key 512]
            kTf = sb1.tile([64, 2, KD, S], bf16, tag="kTf")
            qTa64 = sb1.tile([64, 2, KD, TO], bf16, tag="qTa64")
            v1 = sb1.tile([128, 4, H, 65], bf16, tag="v1")          # full keys
            oT = sb1.tile([128, KD, TO], f32r, tag="oT")
            oTb = sb1.tile([128, KD, TO], bf16, tag="oTb")
            h1 = sb1.tile([128, KD, TO], f32r, tag="h1")
            h1_bf = sb1.tile([128, KD, TO], bf16, tag="h1_bf")
            r1 = sb1.tile([128, KD, TO], f32r, tag="r1")
            # y2acc aliases r1: r1's last read (ln_finish LN1) strictly
            # precedes y2acc's first write (W2 phase 0 output).
            y2acc = r1.bitcast(f32)
            u = sb1.tile([128, FH, TO], bf16, tag="u")
            sums16 = sb1.tile([16, TO], f32, tag="sums16")
            recIP = sb1.tile([16, TO], f32r, tag="recIP")

            xTb_r = xTb.rearrange("(ko kp) t -> kp ko t", kp=128)
            nc.sync.dma_start(h[:], xTf.rearrange("(ko kp) t -> kp ko t", kp=128))
            nc.sync.dma_start(h_bf[:], xTb_r[:, :, 0:TO])
            nc.sync.dma_start(hrem[:], xTb_r[:, :, TO:S])
            nc.sync.dma_start(cst_sb[:], cst[:])
            nc.sync.dma_start(crow_sb[:], crow[:])
            nc.sync.dma_start(msk_sb[:], msk[:])
            nc.sync.dma_start(selc_sb[:], selc[:])
            # ones column of v1 (written once; data writes never touch col 64)
            with nc.allow_non_contiguous_dma(reason="tiny one-time ones-column fill"):
                nc.sync.dma_start(v1[:, :, :, 64], cstb[:])
            nc.gpsimd.memset(sums16[:], 1.0)
            # warm up the collective path (ENCD staging) with a tiny AllReduce
            wrm_in = dram.tile([128, 16], bf16, tag="wrmin")
            nc.gpsimd.dma_start(wrm_in[:], cstb[:, 0:16])
            wrm_out = dram.tile([128, 16], bf16, tag="wrmout")
            nc.gpsimd.collective_compute(
                "AllReduce", OP.add, replica_groups=RG,
                ins=[wrm_in.opt()], outs=[wrm_out.opt()])

            ones_col = cst_sb[:, 64:65]          # [128,1] f32r, stats lhsT
            onesr_ln = crow_sb[0:1, 0:128]       # [1,128] f32r @p0, LN bcast lhsT

            def proj_pair(wsrc, rhs_h, bias_fn, act_fn):
                """Eight [128,TO] projections, paired two-per-PSUM-bank.

                Weight chunks come from the unified ring (consumption order).
                """
                for t2 in range(KD // 2):
                    wc = wrp.tile([128, 2, KD, 128], wdt, tag="wr")
                    nc.sync.dma_start(wc[:], wsrc(t2))
                    ps = psA.tile([128, 2 * TO], f32, tag="ps")
                    for half in range(2):
                        t = 2 * t2 + half
                        sl = ps[:, half * TO:(half + 1) * TO]
                        for k in range(KD):
                            nc.tensor.matmul(sl, wc[:, half, k, :], rhs_h[:, k, :],
                                             start=(k == 0), stop=(k == KD - 1))
                        act_fn(t, sl, bias_fn(t))

            def ln_begin():
                ps_s = psB.tile([1, TO], f32, tag="aux1", bufs=2, name="ps_s")
                ps_q = psB.tile([1, TO], f32, tag="aux1", bufs=2, name="ps_q")
                return ps_s, ps_q

            def ln_accum(st, o, rsl):
                """Accumulate sum / sum-of-squares of r's o-th tile (PE + DVE)."""
                ps_s, ps_q = st
                sq = sb2.tile([128, TO], f32r, tag="sq")
                with nc.allow_low_precision(reason="LN sq rounding"):
                    nc.vector.tensor_tensor(sq[:], rsl.bitcast(f32), rsl.bitcast(f32),
                                            OP.mult)
                nc.tensor.matmul(ps_s[:], ones_col, rsl, start=(o == 0),
                                 stop=(o == KD - 1))
                nc.tensor.matmul(ps_q[:], ones_col, sq[:], start=(o == 0),
                                 stop=(o == KD - 1))

            def ln_finish(st, r, dst, dst_bf, g_col, be_col, defer=None,
                          mid_hook=None):
                """dst = (r - mean) * rstd * g + be (f32r), dst_bf same in bf16.

                Per-tile: add+mult on DVE, the g/be tensor_scalar on GpSimd
                (idle during LN), and the bf16 copy on Scalar straight from b
                (scale/bias fused) -- three engines pipelined per tile.
                """
                ps_s, ps_q = st
                negm = sb2.tile([1, TO], f32r, tag="negm", bufs=1)
                with nc.allow_low_precision(reason="LN stats rounding"):
                    nc.vector.tensor_scalar(negm[:], ps_s[:], -1.0 / D, None, OP.mult)
                msq = sb2.tile([1, TO], f32, tag="lnscr", bufs=3)
                nc.vector.tensor_tensor(msq[:], negm[:].bitcast(f32),
                                        negm[:].bitcast(f32), OP.mult)
                var = sb2.tile([1, TO], f32, tag="lnscr", bufs=3)
                # var = E[x^2] - mean^2 (EPS dropped: |var| ~ O(1) >> 1e-6)
                nc.vector.scalar_tensor_tensor(var[:], ps_q[:], 1.0 / D, msq[:],
                                               OP.mult, OP.subtract)
                vrec = sb2.tile([1, TO], f32, tag="lnscr", bufs=3)
                nc.vector.reciprocal_approx_fast(vrec[:], var[:])
                rstd = sb2.tile([1, TO], f32r, tag="rstd", bufs=1)
                with nc.allow_low_precision(reason="LN rstd rounding"):
                    nc.scalar.activation(rstd[:], vrec[:], AF.Sqrt)
                bcast = psB.tile([128, 2 * TO], f32, tag="aux", bufs=1, name="bc")
                pnm, prs = bcast[:, 0:TO], bcast[:, TO:2 * TO]
                nc.tensor.matmul(pnm, onesr_ln, negm[:], start=True, stop=True)
                nc.tensor.matmul(prs, onesr_ln, rstd[:], start=True, stop=True)
                for o in range(KD):
                    a = sb2.tile([128, TO], f32, tag="lna")
                    nc.vector.tensor_tensor(a[:], r[:, o, :].bitcast(f32), pnm, OP.add)
                    if defer is None:
                        b = sb2.tile([128, TO], f32, tag="lnb", bufs=3)
                        bsl = b[:]
                    else:
                        # deferred f32r writes: b must outlive the whole apply
                        b = sb2.tile([128, TO], f32, tag="lnb2", bufs=KD)
                        bsl = b[:]
                    nc.vector.tensor_tensor(bsl, a[:], prs, OP.mult)
                    if defer is None:
                        with nc.allow_low_precision(reason="f32r LN output"):
                            nc.gpsimd.tensor_scalar(dst[:, o, :], bsl,
                                                    g_col[:, o, :],
                                                    be_col[:, o, :], OP.mult, OP.add)
                    else:
                        defer.append((o, bsl))
                    nc.scalar.activation(dst_bf[:, o, :], bsl, AF.Identity,
                                         bias=be_col[:, o, :],
                                         scale=g_col[:, o, :])
                    if mid_hook is not None and o == KD // 2 - 1:
                        mid_hook()

            def load_kv_w(l):
                wk, wv = [], []
                for t2 in range(KD // 2):
                    wc = wkp.tile([128, 2, KD, 128], wdt, tag="wk")
                    nc.sync.dma_start(wc[:], WkT[l, t2])
                    wk.append(wc)
                for nq in range(4):
                    wv_s = wvp.tile([128, KD, 256], wdt, tag="wv")
                    nc.sync.dma_start(wv_s[:], WvN[l, nq])
                    wv.append(wv_s)
                return wk, wv

            hg_out_prev = None
            kvw = load_kv_w(0)
            for l in range(nlayers):
                # ---- per-layer bias/gain staging (host-packed) ----
                bia = sb2.tile([128, KD, 9], f32, tag="bias")
                nc.gpsimd.dma_start(bia[:], bias9[l])
                b1_sb = sb2.tile([128, FT, 1], f32, tag="b1")
                nc.gpsimd.dma_start(b1_sb[:], b1h[l])
                # unpack the AllReduce halves: hrem = hsum - h_own (staging
                # DMAs live on the scalar queue now, so the blocking hsum
                # triggers can't starve anything latency-critical on gpsimd)
                if l > 0:
                    for i in range(2):
                        hs = sb2.tile([128, KD // 2, TO], bf16, tag="hsum", bufs=2)
                        nc.gpsimd.dma_start(hs[:], hg_out_prev[i][:])
                        sl2 = slice(i * (KD // 2), (i + 1) * (KD // 2))
                        nc.vector.tensor_tensor(hrem[:, sl2, :], hs[:],
                                                h_bf[:, sl2, :], OP.subtract)

                # ============ K/V own-half projections (no collective dep) ======
                wkc, wvc = kvw
                for t2 in range(KD // 2):
                    ps = psA.tile([128, 2 * TO], f32, tag="ps")
                    kst = sb2.tile([128, 2 * TO], bf16, tag="kst", bufs=2)
                    for half in range(2):
                        t = 2 * t2 + half
                        sl = ps[:, half * TO:(half + 1) * TO]
                        for k in range(KD):
                            nc.tensor.matmul(sl, wkc[t2][:, half, k, :],
                                             h_bf[:, k, :],
                                             start=(k == 0), stop=(k == KD - 1))
                        nc.scalar.activation(kst[:, half * TO:(half + 1) * TO], sl,
                                             AF.Identity, bias=bia[:, t, 1:2])
                    for hh in range(2):
                        nc.scalar.dma_start(
                            kTf[:, hh, 2 * t2:2 * t2 + 2, 0:TO],
                            kst[hh * 64:(hh + 1) * 64, :])
                def v_own(nq):
                    pv = psA.tile([128, 2 * TO], f32, tag="ps")
                    for tt in range(2):
                        sl = pv[:, tt * 256:(tt + 1) * 256]
                        for k in range(KD):
                            nc.tensor.matmul(sl, h_bf[:, k, tt * 128:(tt + 1) * 128],
                                             wvc[nq][:, k, :], start=(k == 0),
                                             stop=(k == KD - 1))
                        nc.vector.tensor_copy(v1[:, tt, nq * 4:(nq + 1) * 4, 0:64],
                                              sl)
                v_own(0)

                # ================= Q projection =================================
                def q_act(t, sl, b):
                    qst = sb2.tile([128, TO], bf16, tag="qst", bufs=2)
                    nc.scalar.activation(qst[:], sl, AF.Identity, bias=b)
                    for hh in range(2):
                        nc.scalar.dma_start(qTa64[:, hh, t, :],
                                            qst[hh * 64:(hh + 1) * 64, :])
                proj_pair(lambda t2: WqT[l, t2], h_bf,
                          lambda t: bia[:, t, 0:1], q_act)

                # ============ QK on local key blocks (no collective dep) ========
                ea_tiles = {}
                for t in range(KD):
                    for kb in range(2):
                        lt = psA.tile([128, 2 * TO], f32, tag="ps")
                        for hh in range(2):
                            nc.tensor.matmul(lt[:, hh * TO:(hh + 1) * TO],
                                             kTf[:, hh, t, kb * 128:(kb + 1) * 128],
                                             qTa64[:, hh, t, :], start=True,
                                             stop=True)
                        ea = eap.tile([128, 2 * TO], bf16, tag="ea")
                        nc.scalar.activation(ea[:], lt[:], AF.Exp,
                                             bias=msk_sb[:, kb:kb + 1],
                                             scale=0.125)
                        ea_tiles[(t, kb)] = ea
                    if t in (1, 3, 5):
                        # fill the exp-paced QK-local window with V-own work
                        v_own(1 + t // 2)

                # ============ K peer-half projections (need hrem) ===============
                for t2 in range(KD // 2):
                    ps = psA.tile([128, 2 * TO], f32, tag="ps")
                    kst = sb2.tile([128, 2 * TO], bf16, tag="kst", bufs=2)
                    for half in range(2):
                        t = 2 * t2 + half
                        sl = ps[:, half * TO:(half + 1) * TO]
                        for k in range(KD):
                            nc.tensor.matmul(sl, wkc[t2][:, half, k, :],
                                             hrem[:, k, :],
                                             start=(k == 0), stop=(k == KD - 1))
                        nc.scalar.activation(kst[:, half * TO:(half + 1) * TO], sl,
                                             AF.Identity, bias=bia[:, t, 1:2])
                    for hh in range(2):
                        nc.scalar.dma_start(
                            kTf[:, hh, 2 * t2:2 * t2 + 2, TO:S],
                            kst[hh * 64:(hh + 1) * 64, :])
                # ============ V peer-half projections (need hrem) ===============
                for nq in range(4):
                    pv = psA.tile([128, 2 * TO], f32, tag="ps")
                    for tt in range(2):
                        sl = pv[:, tt * 256:(tt + 1) * 256]
                        for k in range(KD):
                            nc.tensor.matmul(sl, hrem[:, k, tt * 128:(tt + 1) * 128],
                                             wvc[nq][:, k, :], start=(k == 0),
                                             stop=(k == KD - 1))
                        nc.vector.tensor_copy(v1[:, 2 + tt, nq * 4:(nq + 1) * 4, 0:64],
                                              sl)

                # ---- prefetch next layer's K/V weight chunks (sync idle now) ---
                if l + 1 < nlayers:
                    kvw = load_kv_w(l + 1)

                # ============ QK remote + AV per d-tile =========================
                def attn_norm(t):
                    """Per-head-pair softmax normalize + bv bias for d-tile t."""
                    if t % 2 == 0:
                        # refresh the full-16-row recip; rows through 2t+3 are
                        # already valid (lag-1: AV(t+1) done), covering t, t+1.
                        recF = sb2.tile([16, TO], f32, tag="recF", bufs=1)
                        nc.vector.reciprocal_approx_fast(recF[:], sums16[:])
                        with nc.allow_low_precision(reason="softmax recip round"):
                            nc.vector.tensor_scalar(recIP[:], recF[:], 1.0, None,
                                                    OP.mult)
                    prb = psB.tile([128, 2 * TO], f32, tag="aux", bufs=1, name="prb")
                    # contract only over written recIP rows (selc rows >2t+3 are 0)
                    nr = min(2 * t + 4, 16)
                    nc.tensor.matmul(prb[:, 0:TO],
                                     selc_sb[0:nr, t * 128:(t + 1) * 128],
                                     recIP[0:nr, :], start=True, stop=True)
                    with nc.allow_low_precision(reason="f32r attn normalize"):
                        nc.vector.tensor_tensor(oT[:, t, :], oT[:, t, :].bitcast(f32),
                                                prb[:, 0:TO], OP.mult)
                    nc.scalar.activation(oTb[:, t, :], oT[:, t, :], AF.Identity,
                                         bias=bia[:, t, 2:3])

                for t in range(KD):
                    for kb in range(2, 4):
                        lt = psA.tile([128, 2 * TO], f32, tag="ps")
                        for hh in range(2):
                            nc.tensor.matmul(lt[:, hh * TO:(hh + 1) * TO],
                                             kTf[:, hh, t, kb * 128:(kb + 1) * 128],
                                             qTa64[:, hh, t, :], start=True, stop=True)
                        ea = sb2.tile([128, 2 * TO], bf16, tag="ear", bufs=4)
                        nc.scalar.activation(ea[:], lt[:], AF.Exp,
                                             bias=msk_sb[:, kb:kb + 1], scale=0.125)
                        ea_tiles[(t, kb)] = ea
                    po = psB.tile([65, 2 * TO], f32, tag="po")
                    for pi in range(2):
                        for kb in range(4):
                            nc.tensor.matmul(po[:, pi * TO:(pi + 1) * TO],
                                             v1[:, kb, 2 * t + pi, :],
                                             ea_tiles[(t, kb)][:, pi * TO:(pi + 1) * TO],
                                             start=(kb == 0), stop=(kb == 3))
                    ov = sb2.tile([65, 2 * TO], f32, tag="ov")
                    nc.vector.tensor_copy(ov[:], po[:])
                    for pi in range(2):
                        nc.gpsimd.dma_start(
                            oT[pi * 64:pi * 64 + 64, t, :].bitcast(f32),
                            ov[0:64, pi * TO:(pi + 1) * TO])
                    nc.gpsimd.dma_start(sums16[2 * t:2 * t + 2, :], ov[64:65, :])
                    if t > 0:   # lag-1: normalize previous tile, PE queue stays warm
                        attn_norm(t - 1)
                attn_norm(KD - 1)

                # ============== Wo + residual + LN1 stats (interleaved) =========
                st1 = ln_begin()

                def wo_act(m, sl, b):
                    at = sb2.tile([128, TO], f32, tag="att")
                    nc.scalar.activation(at[:], sl, AF.Identity, bias=b)
                    with nc.allow_low_precision(reason="f32r residual"):
                        nc.vector.tensor_tensor(r1[:, m, :], at[:],
                                                h[:, m, :].bitcast(f32), OP.add)
                    ln_accum(st1, m, r1[:, m, :])
                proj_pair(lambda m2: WoT[l, m2], oTb,
                          lambda m: bia[:, m, 3:4], wo_act)

                ln_finish(st1, r1, h1, h1_bf, bia[:, :, 5:6],
                          bia[:, :, 6:7])

                # ============== FFN (two F-halves) + LN2 stats ==================
                st2 = ln_begin()
                for ph in range(2):
                    for fq in range(FH // 2):   # 2 f-tiles per ring chunk
                        wc1 = wrp.tile([128, 2, KD, 128], wdt, tag="wr")
                        nc.sync.dma_start(wc1[:], W1T[l, ph * (FH // 2) + fq])
                        pu = psA.tile([128, 2 * TO], f32, tag="ps")
                        for half in range(2):
                            fu = 2 * fq + half
                            fg = ph * FH + fu
                            sl = pu[:, half * TO:(half + 1) * TO]
                            for k in range(KD):
                                nc.tensor.matmul(sl, wc1[:, half, k, :],
                                                 h1_bf[:, k, :],
                                                 start=(k == 0),
                                                 stop=(k == KD - 1))
                            nc.scalar.activation(u[:, fu, :], sl, AF.Relu,
                                                 bias=b1_sb[:, fg, 0:1])
                    for m2 in range(KD // 2):
                        py = psA.tile([128, 2 * TO], f32, tag="ps")
                        for half in range(2):
                            m = 2 * m2 + half
                            wc2 = wrp.tile([128, FH, 128], wdt, tag="wr")
                            nc.sync.dma_start(wc2[:], W2T[l, ph, m])
                            sl = py[:, half * TO:(half + 1) * TO]
                            for fo in range(FH):
                                nc.tensor.matmul(sl, wc2[:, fo, :], u[:, fo, :],
                                                 start=(fo == 0), stop=(fo == FH - 1))
                            if ph == 0:
                                nc.scalar.activation(y2acc[:, m, :].bitcast(f32r),
                                                     sl, AF.Identity,
                                                     bias=bia[:, m, 4:5])
                            else:
                                tmp = sb2.tile([128, TO], f32, tag="att")
                                nc.vector.tensor_tensor(tmp[:], sl,
                                                        h1[:, m, :].bitcast(f32),
                                                        OP.add)
                                with nc.allow_low_precision(reason="f32r residual"):
                                    nc.vector.tensor_tensor(
                                        y2acc[:, m, :].bitcast(f32r), tmp[:],
                                        y2acc[:, m, :], OP.add)
                                ln_accum(st2, m, y2acc[:, m, :].bitcast(f32r))
                r2 = y2acc[:].bitcast(f32r)

                # LN2's f32r (gpsimd) writes are deferred so the h_bf pack
                # and the collective trigger go first on the gpsimd queue.
                # The AllReduce is split in two halves; the first is triggered
                # the moment h_bf tiles 0..3 exist (mid-apply), pipelining the
                # two mesh ops so hrem's first half lands earlier next layer.
                dts = []
                hg_out_prev = [None, None]

                def cc_half(i):
                    hg_in = dram.tile([128, KD // 2 * TO], bf16, tag=f"hgin{i}")
                    nc.gpsimd.dma_start(hg_in[:],
                                        h_bf[:, i * (KD // 2):(i + 1) * (KD // 2), :])
                    hg_o = dram.tile([128, KD // 2 * TO], bf16, tag=f"hgout{i}")
                    nc.gpsimd.collective_compute(
                        "AllReduce", OP.add, replica_groups=RG,
                        ins=[hg_in.opt()], outs=[hg_o.opt()])
                    hg_out_prev[i] = hg_o

                do_cc = use_cc and l < nlayers - 1
                ln_finish(st2, r2, h, h_bf, bia[:, :, 7:8],
                          bia[:, :, 8:9], defer=dts,
                          mid_hook=(lambda: cc_half(0)) if do_cc else None)

                if do_cc:
                    cc_half(1)
                    for o, bsl in dts:
                        with nc.allow_low_precision(reason="f32r LN output"):
                            nc.gpsimd.tensor_scalar(h[:, o, :], bsl,
                                                    bia[:, o, 7:8],
                                                    bia[:, o, 8:9], OP.mult, OP.add)
                else:
                    for o, bsl in dts:
                        with nc.allow_low_precision(reason="f32r LN output"):
                            nc.vector.tensor_scalar(h[:, o, :], bsl,
                                                    bia[:, o, 7:8],
                                                    bia[:, o, 8:9], OP.mult, OP.add)
                    if not use_cc and l < nlayers - 1:
                        # no-cc debug path: "sum" = 2*h so hrem = h (self-pair)
                        hg_out_prev = dram.tile([128, KD * TO], bf16, tag="hgout")
                        hdub = sb2.tile([128, KD, TO], bf16, tag="hsum", bufs=1,
                                        name="hdub")
                        nc.vector.tensor_scalar(hdub[:], h_bf[:], 2.0, None, OP.mult)
                        nc.sync.dma_start(hg_out_prev[:],
                                          hdub[:].rearrange("p a b -> p (a b)"))

            nc.sync.dma_start(out.rearrange("(ko kp) t -> kp ko t", kp=128),
                              h[:].bitcast(f32))

    nc.compile()
    return nc


def _selc():
    sel = np.zeros((16, KD * 128), np.float32)
    for t in range(KD):
        for m in range(128):
            sel[2 * t + m // 64, t * 128 + m] = 1.0
    return sel


def _pos_encoding(position, d):
    pos = np.arange(position)[:, None].astype(np.float32)
    i = np.arange(d)[None, :].astype(np.float32)
    angle = pos / np.power(10000.0, 2.0 * np.floor(i / 2.0) / np.float32(d))
    angle[:, 0::2] = np.sin(angle[:, 0::2])
    angle[:, 1::2] = np.cos(angle[:, 1::2])
    return angle.astype(np.float32)  # [position, d]


def _get_nc():
    if "nc" not in _cache:
        _cache["nc"] = build()
    return _cache["nc"]


def prepare_in_maps(inputs):
    """Host-side prep: pack weights/biases, shard tokens across 8 cores."""
    import ml_dtypes
    bf = ml_dtypes.bfloat16 if WBF else np.float32
    inp = {k: np.asarray(v, dtype=np.float32) for k, v in inputs.items()}
    pe = _pos_encoding(MAX_POS, D)[:S]
    x = inp["x"] + pe[None]

    def lhsT_pack(w):  # [L, D, D] -> [L, t2, kp, a, k, col]
        return np.ascontiguousarray(
            w.reshape(L, KD, 128, KD // 2, 2, 128).transpose(0, 3, 2, 4, 1, 5)
        ).astype(bf)

    common = {
        "WqT": lhsT_pack(inp["Wq"]),
        "WkT": lhsT_pack(inp["Wk"]),
        "WoT": lhsT_pack(inp["Wo"]),
        "WvN": np.ascontiguousarray(
            inp["Wv"].reshape(L, KD, 128, 4, 256).transpose(0, 3, 2, 1, 4)).astype(bf),
        "W1T": np.ascontiguousarray(
            inp["W1"].reshape(L, KD, 128, FT // 2, 2, 128)
            .transpose(0, 3, 2, 4, 1, 5)).astype(bf),
        "W2T": np.ascontiguousarray(
            inp["W2"].reshape(L, 2, FH, 128, KD, 128)
            .transpose(0, 1, 4, 3, 2, 5)).astype(bf),
    }
    pk = lambda a: np.ascontiguousarray(a.reshape(L, KD, 128).transpose(0, 2, 1))
    common["bias9"] = np.ascontiguousarray(np.stack(
        [pk(inp[k]) for k in ["bq", "bk", "bv", "bo", "b2", "g1", "be1", "g2", "be2"]],
        axis=-1))
    common["b1h"] = np.ascontiguousarray(
        inp["b1"].reshape(L, FT, 128).transpose(0, 2, 1)[..., None])
    common["cst"] = np.ones((128, 65), np.float32)
    common["cstb"] = np.ones((128, 64), ml_dtypes.bfloat16)
    common["crow"] = np.ones((65, 256), np.float32)
    common["selc"] = _selc()
    in_maps = []
    for c in range(NCORES):
        b, r = c // 2, c % 2
        m = dict(common)
        own = x[b, r * TO:(r + 1) * TO, :]
        oth = x[b, (1 - r) * TO:(2 - r) * TO, :]
        m["xTf"] = np.ascontiguousarray(own.T)
        m["xTb"] = np.ascontiguousarray(
            np.concatenate([own, oth], axis=0).T.astype(ml_dtypes.bfloat16))
        mk = (inp["mask"][b, 0, 0] * np.float32(-1e9)).astype(np.float32)
        mk4 = mk.reshape(4, 128)
        order = [2 * r, 2 * r + 1, 2 * (1 - r), 2 * (1 - r) + 1]
        m["msk"] = np.ascontiguousarray(mk4[order].T)
        in_maps.append(m)
    return in_maps


def kernel(**inputs):
    _, _, _, _, run_bass_kernel_spmd = _imports()
    nc = _get_nc()
    in_maps = prepare_in_maps(inputs)
    res = run_bass_kernel_spmd(nc, in_maps, core_ids=list(range(NCORES)))
    B = np.asarray(inputs["x"]).shape[0]
    out = np.stack([
        np.concatenate([res.results[2 * b]["out"].T,
                        res.results[2 * b + 1]["out"].T], axis=0)
        for b in range(B)])
    return out.astype(np.float32)


# revision 55
# speedup vs baseline: 1.0237x; 1.0237x over previous
"""Trainium2 Bass kernel for a 6-layer transformer encoder.

Problem: B=4, S=512, D=1024, H=16 heads (depth 64), F=4096, L=6 layers, fp32.

Sharding: sequence-sharding within core pairs. Core c handles batch b=c//2,
token half r=c%2. Each core computes Q/attention/Wo/FFN/LN for its own 256
tokens, but K and V for ALL 512 tokens: the own half directly from its own
hidden state (no communication), the peer half from hrem = AllReduce_sum(h)
- h_own. The AllReduce (bf16, 0.5MB) is triggered the moment LN2 finishes a
layer, and the first instruction that depends on it sits behind ~30us of
guaranteed-local PE work (K_own/V_own/Q/QK_local), so the PE never idles
waiting for the collective. Keys are kept in rank-relative order (own 256
first, peer 256 second) so the SPMD program is rank-uniform; softmax is
permutation-invariant over keys so this is exact. The mask and x are packed
host-side in the same rank-relative order.

Precision: weights are pre-cast to bf16 host-side; matmul operands are bf16,
the residual stream (h, r1, h1, y2acc) stays f32r. Softmax uses exp without
max-subtraction (logits are O(1)), row sums via a ones-column appended to V.

Scheduling notes:
- The PE p-state ramps (0.65 -> 1.2 -> 2.4 GHz after 3us continuous busy);
  every idle gap resets it, so phases are ordered to keep one long stream.
- Weights stream through a single 6-buffer ring fed on the sync queue in
  exact consumption order (Q, Wo, W1, W2); Wk/Wv chunks are prefetched one
  layer ahead into held pools during the previous layer's attention, so no
  phase ever waits on a weight chunk.
- Queue discipline: sync = bulk weights only (must never block); scalar
  (hwdge) = latency-critical staging DMAs, each trigger directly after its
  producing activation so it never stalls the queue; gpsimd (swdge, slow) =
  slack-tolerant transfers (oT/sums, hsum unpack, pack+collective). The
  blocking hsum unpack is placed after QK-local's staging on the in-order
  gpsimd queue.
- The AllReduce is split into two pipelined 0.25MB halves, the first
  triggered mid-LN2-apply; LN2's f32r tensor_scalar writes are deferred
  (vector engine) until after the collective trigger.
- Softmax normalize is lag-1 per d-tile (reciprocal_approx_fast every other
  tile); QK-local exps are issued before any AV so exp latency hides behind
  further matmuls.
- PSUM: matmul outputs paired two-per-bank ([*,512] tiles); never mix
  operand base partitions within one bank (matmul operands must sit at base
  partition 0 - base-64 lhsT/rhs faults), never interleave two open
  accumulation groups in one bank. PSUM budget: psA 3 + po 2 + aux 1 +
  stats 2 = 8 banks.
"""

import numpy as np

TO = 256        # own tokens per core
S = 512         # total keys per batch element
D = 1024        # model dim
KD = D // 128   # 8 d-tiles
H = 16          # heads
DH = 64         # head dim
F = 4096        # ff dim
FT = F // 128   # 32 f-tiles
FH = FT // 2    # f-tiles per FFN half
L = 6           # layers
EPS = 1e-6
MAX_POS = 1000
NCORES = 8
WBF = True      # bf16 weights + bf16 matmul operands

_cache = {}


def _imports():
    import sys
    try:
        import concourse.bass  # noqa
    except ImportError:
        for p in ("/opt/trn_rl_repo", "/root/.axon_site/_ro/trn_rl_repo"):
            if p not in sys.path:
                sys.path.insert(0, p)
    import concourse.bass as bass
    import concourse.mybir as mybir
    import concourse.tile as tile
    from concourse import bacc
    from concourse.bass_utils import run_bass_kernel_spmd
    return bass, mybir, tile, bacc, run_bass_kernel_spmd


def build(nlayers=L, use_cc=True, debug=False):
    bass, mybir, tile, bacc, _ = _imports()
    f32 = mybir.dt.float32
    f32r = mybir.dt.float32r
    bf16 = mybir.dt.bfloat16
    AF = mybir.ActivationFunctionType
    OP = mybir.AluOpType
    RG = [[0, 1], [2, 3], [4, 5], [6, 7]]

    nc = bacc.Bacc(None, target_bir_lowering=False, debug=True, num_devices=8)

    # ---- kernel I/O ----
    wdt = bf16 if WBF else f32r
    xTf = nc.declare_dram_parameter("xTf", [D, TO], f32r, isOutput=False)
    xTb = nc.declare_dram_parameter("xTb", [D, S], bf16, isOutput=False)
    msk = nc.declare_dram_parameter("msk", [128, 4], f32, isOutput=False)
    # lhsT-packed: [l, t, kp, k, col] = W[l, k*128+kp, t*128+col]
    WqT = nc.declare_dram_parameter("WqT", [L, KD // 2, 128, 2, KD, 128], wdt,
                                    isOutput=False)
    WkT = nc.declare_dram_parameter("WkT", [L, KD // 2, 128, 2, KD, 128], wdt,
                                    isOutput=False)
    WoT = nc.declare_dram_parameter("WoT", [L, KD // 2, 128, 2, KD, 128], wdt,
                                    isOutput=False)
    # rhs-packed V: [l, nq, kp, k, col] = Wv[l, k*128+kp, nq*256+col]
    WvN = nc.declare_dram_parameter("WvN", [L, 4, 128, KD, 256], wdt, isOutput=False)
    # [l, c, kp, j, k, col] = W1[l, k*128+kp, (2c+j)*128+col]
    W1T = nc.declare_dram_parameter("W1T", [L, FT // 2, 128, 2, KD, 128], wdt,
                                    isOutput=False)
    # [l, ph, m, fp, fo, col] = W2[l, ph*2048 + fo*128+fp, m*128+col]
    W2T = nc.declare_dram_parameter("W2T", [L, 2, KD, 128, FH, 128], wdt,
                                    isOutput=False)
    bias9 = nc.declare_dram_parameter("bias9", [L, 128, KD, 9], f32, isOutput=False)
    b1h = nc.declare_dram_parameter("b1h", [L, 128, FT, 1], f32, isOutput=False)
    cst = nc.declare_dram_parameter("cst", [128, 65], f32r, isOutput=False)   # ones
    cstb = nc.declare_dram_parameter("cstb", [128, 64], bf16, isOutput=False)  # ones
    crow = nc.declare_dram_parameter("crow", [65, 256], f32r, isOutput=False)  # ones
    selc = nc.declare_dram_parameter("selc", [16, KD * 128], f32r, isOutput=False)
    out = nc.declare_dram_parameter("out", [D, TO], f32, isOutput=True)

    with tile.TileContext(nc) as tc:
        with tc.tile_pool(name="sb", bufs=1) as sb1, \
             tc.tile_pool(name="sb2", bufs=2) as sb2, \
             tc.tile_pool(name="wr", bufs=6) as wrp, \
             tc.tile_pool(name="wkp", bufs=4) as wkp, \
             tc.tile_pool(name="wvp", bufs=4) as wvp, \
             tc.tile_pool(name="eap", bufs=16) as eap, \
             tc.tile_pool(name="dram", bufs=2, space="DRAM") as dram, \
             tc.tile_pool(name="psA", bufs=3, space="PSUM") as psA, \
             tc.tile_pool(name="psB", bufs=2, space="PSUM") as psB:

            # ---- persistent tiles ----
            h = sb1.tile([128, KD, TO], f32r, tag="h")
            h_bf = sb1.tile([128, KD, TO], bf16, tag="h_bf")
            hrem = sb1.tile([128, KD, TO], bf16, tag="hrem")
            cst_sb = sb1.tile([128, 65], f32r, tag="cst")
            crow_sb = sb1.tile([65, 256], f32r, tag="crow")
            msk_sb = sb1.tile([128, 4], f32, tag="msk")
            selc_sb = sb1.tile([16, KD * 128], f32r, tag="selc")
            # K for attention: [depth 64, head-in-pair 2, d-tile 8, key 512]
            kTf = sb1.tile([64, 2, KD, S], bf16, tag="kTf")
            qTa64 = sb1.tile([64, 2, KD, TO], bf16, tag="qTa64")
            v1 = sb1.tile([128, 4, H, 65], bf16, tag="v1")          # full keys
            oT = sb1.tile([128, KD, TO], f32r, tag="oT")
            oTb = sb1.tile([128, KD, TO], bf16, tag="oTb")
            h1 = sb1.tile([128, KD, TO], f32r, tag="h1")
            h1_bf = sb1.tile([128, KD, TO], bf16, tag="h1_bf")
            r1 = sb1.tile([128, KD, TO], f32r, tag="r1")
            # y2acc aliases r1: r1's last read (ln_finish LN1) strictly
            # precedes y2acc's first write (W2 phase 0 output).
            y2acc = r1.bitcast(f32)
            u = sb1.tile([128, FH, TO], bf16, tag="u")
            sums16 = sb1.tile([16, TO], f32, tag="sums16")
            recIP = sb1.tile([16, TO], f32r, tag="recIP")

            xTb_r = xTb.rearrange("(ko kp) t -> kp ko t", kp=128)
            nc.sync.dma_start(h[:], xTf.rearrange("(ko kp) t -> kp ko t", kp=128))
            nc.sync.dma_start(h_bf[:], xTb_r[:, :, 0:TO])
            nc.sync.dma_start(hrem[:], xTb_r[:, :, TO:S])
            nc.sync.dma_start(cst_sb[:], cst[:])
            nc.sync.dma_start(crow_sb[:], crow[:])
            nc.sync.dma_start(msk_sb[:], msk[:])
            nc.sync.dma_start(selc_sb[:], selc[:])
            # ones column of v1 (written once; data writes never touch col 64)
            with nc.allow_non_contiguous_dma(reason="tiny one-time ones-column fill"):
                nc.sync.dma_start(v1[:, :, :, 64], cstb[:])
            nc.gpsimd.memset(sums16[:], 1.0)
            # warm up the collective path (ENCD staging) with a tiny AllReduce
            wrm_in = dram.tile([128, 16], bf16, tag="wrmin")
            nc.gpsimd.dma_start(wrm_in[:], cstb[:, 0:16])
            wrm_out = dram.tile([128, 16], bf16, tag="wrmout")
            nc.gpsimd.collective_compute(
                "AllReduce", OP.add, replica_groups=RG,
                ins=[wrm_in.opt()], outs=[wrm_out.opt()])

            ones_col = cst_sb[:, 64:65]          # [128,1] f32r, stats lhsT
            onesr_ln = crow_sb[0:1, 0:128]       # [1,128] f32r @p0, LN bcast lhsT

            def proj_pair(wsrc, rhs_h, bias_fn, act_fn):
                """Eight [128,TO] projections, paired two-per-PSUM-bank.

                Weight chunks come from the unified ring (consumption order).
                """
                for t2 in range(KD // 2):
                    wc = wrp.tile([128, 2, KD, 128], wdt, tag="wr")
                    nc.sync.dma_start(wc[:], wsrc(t2))
                    ps = psA.tile([128, 2 * TO], f32, tag="ps")
                    for half in range(2):
                        t = 2 * t2 + half
                        sl = ps[:, half * TO:(half + 1) * TO]
                        for k in range(KD):
                            nc.tensor.matmul(sl, wc[:, half, k, :], rhs_h[:, k, :],
                                             start=(k == 0), stop=(k == KD - 1))
                        act_fn(t, sl, bias_fn(t))

            def ln_begin():
                ps_s = psB.tile([1, TO], f32, tag="aux1", bufs=2, name="ps_s")
                ps_q = psB.tile([1, TO], f32, tag="aux1", bufs=2, name="ps_q")
                return ps_s, ps_q

            def ln_accum(st, o, rsl):
                """Accumulate sum / sum-of-squares of r's o-th tile (PE + DVE)."""
                ps_s, ps_q = st
                sq = sb2.tile([128, TO], f32r, tag="sq")
                with nc.allow_low_precision(reason="LN sq rounding"):
                    nc.vector.tensor_tensor(sq[:], rsl.bitcast(f32), rsl.bitcast(f32),
                                            OP.mult)
                nc.tensor.matmul(ps_s[:], ones_col, rsl, start=(o == 0),
                                 stop=(o == KD - 1))
                nc.tensor.matmul(ps_q[:], ones_col, sq[:], start=(o == 0),
                                 stop=(o == KD - 1))

            def ln_finish(st, r, dst, dst_bf, g_col, be_col, defer=None,
                          mid_hook=None):
                """dst = (r - mean) * rstd * g + be (f32r), dst_bf same in bf16.

                Per-tile: add+mult on DVE, the g/be tensor_scalar on GpSimd
                (idle during LN), and the bf16 copy on Scalar straight from b
                (scale/bias fused) -- three engines pipelined per tile.
                """
                ps_s, ps_q = st
                negm = sb2.tile([1, TO], f32r, tag="negm", bufs=1)
                with nc.allow_low_precision(reason="LN stats rounding"):
                    nc.vector.tensor_scalar(negm[:], ps_s[:], -1.0 / D, None, OP.mult)
                msq = sb2.tile([1, TO], f32, tag="lnscr", bufs=3)
                nc.vector.tensor_tensor(msq[:], negm[:].bitcast(f32),
                                        negm[:].bitcast(f32), OP.mult)
                var = sb2.tile([1, TO], f32, tag="lnscr", bufs=3)
                # var = E[x^2] - mean^2 (EPS dropped: |var| ~ O(1) >> 1e-6)
                nc.vector.scalar_tensor_tensor(var[:], ps_q[:], 1.0 / D, msq[:],
                                               OP.mult, OP.subtract)
                vrec = sb2.tile([1, TO], f32, tag="lnscr", bufs=3)
                nc.vector.reciprocal_approx_fast(vrec[:], var[:])
                rstd = sb2.tile([1, TO], f32r, tag="rstd", bufs=1)
                with nc.allow_low_precision(reason="LN rstd rounding"):
                    nc.scalar.activation(rstd[:], vrec[:], AF.Sqrt)
                bcast = psB.tile([128, 2 * TO], f32, tag="aux", bufs=1, name="bc")
                pnm, prs = bcast[:, 0:TO], bcast[:, TO:2 * TO]
                nc.tensor.matmul(pnm, onesr_ln, negm[:], start=True, stop=True)
                nc.tensor.matmul(prs, onesr_ln, rstd[:], start=True, stop=True)
                for o in range(KD):
                    a = sb2.tile([128, TO], f32, tag="lna")
                    nc.vector.tensor_tensor(a[:], r[:, o, :].bitcast(f32), pnm, OP.add)
                    if defer is None:
                        b = sb2.tile([128, TO], f32, tag="lnb", bufs=3)
                        bsl = b[:]
                    else:
                        # deferred f32r writes: b must outlive the whole apply
                        b = sb2.tile([128, TO], f32, tag="lnb2", bufs=KD)
                        bsl = b[:]
                    nc.vector.tensor_tensor(bsl, a[:], prs, OP.mult)
                    if defer is None:
                        with nc.allow_low_precision(reason="f32r LN output"):
                            nc.gpsimd.tensor_scalar(dst[:, o, :], bsl,
                                                    g_col[:, o, :],
                                                    be_col[:, o, :], OP.mult, OP.add)
                    else:
                        defer.append((o, bsl))
                    nc.scalar.activation(dst_bf[:, o, :], bsl, AF.Identity,
                                         bias=be_col[:, o, :],
                                         scale=g_col[:, o, :])
                    if mid_hook is not None and o == KD // 2 - 1:
                        mid_hook()

            def load_kv_w(l):
                wk, wv = [], []
                for t2 in range(KD // 2):
                    wc = wkp.tile([128, 2, KD, 128], wdt, tag="wk")
                    nc.sync.dma_start(wc[:], WkT[l, t2])
                    wk.append(wc)
                for nq in range(4):
                    wv_s = wvp.tile([128, KD, 256], wdt, tag="wv")
                    nc.sync.dma_start(wv_s[:], WvN[l, nq])
                    wv.append(wv_s)
                return wk, wv

            hg_out_prev = None
            kvw = load_kv_w(0)
            for l in range(nlayers):
                # ---- per-layer bias/gain staging (host-packed) ----
                bia = sb2.tile([128, KD, 9], f32, tag="bias")
                nc.gpsimd.dma_start(bia[:], bias9[l])
                b1_sb = sb2.tile([128, FT, 1], f32, tag="b1")
                nc.gpsimd.dma_start(b1_sb[:], b1h[l])

                # ============ K/V own-half projections (no collective dep) ======
                wkc, wvc = kvw
                for t2 in range(KD // 2):
                    ps = psA.tile([128, 2 * TO], f32, tag="ps")
                    kst = sb2.tile([128, 2 * TO], bf16, tag="kst", bufs=2)
                    for half in range(2):
                        t = 2 * t2 + half
                        sl = ps[:, half * TO:(half + 1) * TO]
                        for k in range(KD):
                            nc.tensor.matmul(sl, wkc[t2][:, half, k, :],
                                             h_bf[:, k, :],
                                             start=(k == 0), stop=(k == KD - 1))
                        nc.scalar.activation(kst[:, half * TO:(half + 1) * TO], sl,
                                             AF.Identity, bias=bia[:, t, 1:2])
                    for hh in range(2):
                        nc.scalar.dma_start(
                            kTf[:, hh, 2 * t2:2 * t2 + 2, 0:TO],
                            kst[hh * 64:(hh + 1) * 64, :])
                for nq in range(4):
                    pv = psA.tile([128, 2 * TO], f32, tag="ps")
                    for tt in range(2):
                        sl = pv[:, tt * 256:(tt + 1) * 256]
                        for k in range(KD):
                            nc.tensor.matmul(sl, h_bf[:, k, tt * 128:(tt + 1) * 128],
                                             wvc[nq][:, k, :], start=(k == 0),
                                             stop=(k == KD - 1))
                        nc.vector.tensor_copy(v1[:, tt, nq * 4:(nq + 1) * 4, 0:64],
                                              sl)

                # ================= Q projection =================================
                def q_act(t, sl, b):
                    qst = sb2.tile([128, TO], bf16, tag="qst", bufs=2)
                    nc.scalar.activation(qst[:], sl, AF.Identity, bias=b)
                    for hh in range(2):
                        nc.scalar.dma_start(qTa64[:, hh, t, :],
                                            qst[hh * 64:(hh + 1) * 64, :])
                proj_pair(lambda t2: WqT[l, t2], h_bf,
                          lambda t: bia[:, t, 0:1], q_act)

                # ============ QK on local key blocks (no collective dep) ========
                ea_tiles = {}
                for t in range(KD):
                    for kb in range(2):
                        lt = psA.tile([128, 2 * TO], f32, tag="ps")
                        for hh in range(2):
                            nc.tensor.matmul(lt[:, hh * TO:(hh + 1) * TO],
                                             kTf[:, hh, t, kb * 128:(kb + 1) * 128],
                                             qTa64[:, hh, t, :], start=True,
                                             stop=True)
                        ea = eap.tile([128, 2 * TO], bf16, tag="ea")
                        nc.scalar.activation(ea[:], lt[:], AF.Exp,
                                             bias=msk_sb[:, kb:kb + 1],
                                             scale=0.125)
                        ea_tiles[(t, kb)] = ea

                # ---- unpack the AllReduce: hrem = hsum - h_own. The blocking
                # hsum DMA sits here, AFTER the latency-critical qTa64/kTf-own
                # staging on the in-order gpsimd queue. ----
                if l > 0:
                    for i in range(2):
                        hs = sb2.tile([128, KD // 2, TO], bf16, tag="hsum", bufs=2)
                        nc.gpsimd.dma_start(hs[:], hg_out_prev[i][:])
                        sl2 = slice(i * (KD // 2), (i + 1) * (KD // 2))
                        nc.vector.tensor_tensor(hrem[:, sl2, :], hs[:],
                                                h_bf[:, sl2, :], OP.subtract)

                # ============ K peer-half projections (need hrem) ===============
                for t2 in range(KD // 2):
                    ps = psA.tile([128, 2 * TO], f32, tag="ps")
                    kst = sb2.tile([128, 2 * TO], bf16, tag="kst", bufs=2)
                    for half in range(2):
                        t = 2 * t2 + half
                        sl = ps[:, half * TO:(half + 1) * TO]
                        for k in range(KD):
                            nc.tensor.matmul(sl, wkc[t2][:, half, k, :],
                                             hrem[:, k, :],
                                             start=(k == 0), stop=(k == KD - 1))
                        nc.scalar.activation(kst[:, half * TO:(half + 1) * TO], sl,
                                             AF.Identity, bias=bia[:, t, 1:2])
                    for hh in range(2):
                        nc.scalar.dma_start(
                            kTf[:, hh, 2 * t2:2 * t2 + 2, TO:S],
                            kst[hh * 64:(hh + 1) * 64, :])
                # ============ V peer-half projections (need hrem) ===============
                for nq in range(4):
                    pv = psA.tile([128, 2 * TO], f32, tag="ps")
                    for tt in range(2):
                        sl = pv[:, tt * 256:(tt + 1) * 256]
                        for k in range(KD):
                            nc.tensor.matmul(sl, hrem[:, k, tt * 128:(tt + 1) * 128],
                                             wvc[nq][:, k, :], start=(k == 0),
                                             stop=(k == KD - 1))
                        nc.vector.tensor_copy(v1[:, 2 + tt, nq * 4:(nq + 1) * 4, 0:64],
                                              sl)

                # ---- prefetch next layer's K/V weight chunks (sync idle now) ---
                if l + 1 < nlayers:
                    kvw = load_kv_w(l + 1)

                # ============ QK remote + AV per d-tile =========================
                def attn_norm(t):
                    """Per-head-pair softmax normalize + bv bias for d-tile t."""
                    if t % 2 == 0:
                        # refresh the full-16-row recip; rows through 2t+3 are
                        # already valid (lag-1: AV(t+1) done), covering t, t+1.
                        recF = sb2.tile([16, TO], f32, tag="recF", bufs=1)
                        nc.vector.reciprocal_approx_fast(recF[:], sums16[:])
                        with nc.allow_low_precision(reason="softmax recip round"):
                            nc.vector.tensor_scalar(recIP[:], recF[:], 1.0, None,
                                                    OP.mult)
                    prb = psB.tile([128, 2 * TO], f32, tag="aux", bufs=1, name="prb")
                    # contract only over written recIP rows (selc rows >2t+3 are 0)
                    nr = min(2 * t + 4, 16)
                    nc.tensor.matmul(prb[:, 0:TO],
                                     selc_sb[0:nr, t * 128:(t + 1) * 128],
                                     recIP[0:nr, :], start=True, stop=True)
                    with nc.allow_low_precision(reason="f32r attn normalize"):
                        nc.vector.tensor_tensor(oT[:, t, :], oT[:, t, :].bitcast(f32),
                                                prb[:, 0:TO], OP.mult)
                    nc.scalar.activation(oTb[:, t, :], oT[:, t, :], AF.Identity,
                                         bias=bia[:, t, 2:3])

                for t in range(KD):
                    for kb in range(2, 4):
                        lt = psA.tile([128, 2 * TO], f32, tag="ps")
                        for hh in range(2):
                            nc.tensor.matmul(lt[:, hh * TO:(hh + 1) * TO],
                                             kTf[:, hh, t, kb * 128:(kb + 1) * 128],
                                             qTa64[:, hh, t, :], start=True, stop=True)
                        ea = sb2.tile([128, 2 * TO], bf16, tag="ear", bufs=4)
                        nc.scalar.activation(ea[:], lt[:], AF.Exp,
                                             bias=msk_sb[:, kb:kb + 1], scale=0.125)
                        ea_tiles[(t, kb)] = ea
                    po = psB.tile([65, 2 * TO], f32, tag="po")
                    for pi in range(2):
                        for kb in range(4):
                            nc.tensor.matmul(po[:, pi * TO:(pi + 1) * TO],
                                             v1[:, kb, 2 * t + pi, :],
                                             ea_tiles[(t, kb)][:, pi * TO:(pi + 1) * TO],
                                             start=(kb == 0), stop=(kb == 3))
                    ov = sb2.tile([65, 2 * TO], f32, tag="ov")
                    nc.vector.tensor_copy(ov[:], po[:])
                    for pi in range(2):
                        nc.gpsimd.dma_start(
                            oT[pi * 64:pi * 64 + 64, t, :].bitcast(f32),
                            ov[0:64, pi * TO:(pi + 1) * TO])
                    nc.gpsimd.dma_start(sums16[2 * t:2 * t + 2, :], ov[64:65, :])
                    if t > 0:   # lag-1: normalize previous tile, PE queue stays warm
                        attn_norm(t - 1)
                attn_norm(KD - 1)

                # ============== Wo + residual + LN1 stats (interleaved) =========
                st1 = ln_begin()

                def wo_act(m, sl, b):
                    at = sb2.tile([128, TO], f32, tag="att")
                    nc.scalar.activation(at[:], sl, AF.Identity, bias=b)
                    with nc.allow_low_precision(reason="f32r residual"):
                        nc.vector.tensor_tensor(r1[:, m, :], at[:],
                                                h[:, m, :].bitcast(f32), OP.add)
                    ln_accum(st1, m, r1[:, m, :])
                proj_pair(lambda m2: WoT[l, m2], oTb,
                          lambda m: bia[:, m, 3:4], wo_act)

                ln_finish(st1, r1, h1, h1_bf, bia[:, :, 5:6],
                          bia[:, :, 6:7])

                # ============== FFN (two F-halves) + LN2 stats ==================
                st2 = ln_begin()
                for ph in range(2):
                    for fq in range(FH // 2):   # 2 f-tiles per ring chunk
                        wc1 = wrp.tile([128, 2, KD, 128], wdt, tag="wr")
                        nc.sync.dma_start(wc1[:], W1T[l, ph * (FH // 2) + fq])
                        pu = psA.tile([128, 2 * TO], f32, tag="ps")
                        for half in range(2):
                            fu = 2 * fq + half
                            fg = ph * FH + fu
                            sl = pu[:, half * TO:(half + 1) * TO]
                            for k in range(KD):
                                nc.tensor.matmul(sl, wc1[:, half, k, :],
                                                 h1_bf[:, k, :],
                                                 start=(k == 0),
                                                 stop=(k == KD - 1))
                            nc.scalar.activation(u[:, fu, :], sl, AF.Relu,
                                                 bias=b1_sb[:, fg, 0:1])
                    for m2 in range(KD // 2):
                        py = psA.tile([128, 2 * TO], f32, tag="ps")
                        for half in range(2):
                            m = 2 * m2 + half
                            wc2 = wrp.tile([128, FH, 128], wdt, tag="wr")
                            nc.sync.dma_start(wc2[:], W2T[l, ph, m])
                            sl = py[:, half * TO:(half + 1) * TO]
                            for fo in range(FH):
                                nc.tensor.matmul(sl, wc2[:, fo, :], u[:, fo, :],
                                                 start=(fo == 0), stop=(fo == FH - 1))
                            if ph == 0:
                                nc.scalar.activation(y2acc[:, m, :].bitcast(f32r),
                                                     sl, AF.Identity,
                                                     bias=bia[:, m, 4:5])
                            else:
                                tmp = sb2.tile([128, TO], f32, tag="att")
                                nc.vector.tensor_tensor(tmp[:], sl,
                                                        h1[:, m, :].bitcast(f32),
                                                        OP.add)
                                with nc.allow_low_precision(reason="f32r residual"):
                                    nc.vector.tensor_tensor(
                                        y2acc[:, m, :].bitcast(f32r), tmp[:],
                                        y2acc[:, m, :], OP.add)
                                ln_accum(st2, m, y2acc[:, m, :].bitcast(f32r))
                r2 = y2acc[:].bitcast(f32r)

                # LN2's f32r (gpsimd) writes are deferred so the h_bf pack
                # and the collective trigger go first on the gpsimd queue.
                # The AllReduce is split in two halves; the first is triggered
                # the moment h_bf tiles 0..3 exist (mid-apply), pipelining the
                # two mesh ops so hrem's first half lands earlier next layer.
                dts = []
                hg_out_prev = [None, None]

                def cc_half(i):
                    hg_in = dram.tile([128, KD // 2 * TO], bf16, tag=f"hgin{i}")
                    nc.gpsimd.dma_start(hg_in[:],
                                        h_bf[:, i * (KD // 2):(i + 1) * (KD // 2), :])
                    hg_o = dram.tile([128, KD // 2 * TO], bf16, tag=f"hgout{i}")
                    nc.gpsimd.collective_compute(
                        "AllReduce", OP.add, replica_groups=RG,
                        ins=[hg_in.opt()], outs=[hg_o.opt()])
                    hg_out_prev[i] = hg_o

                do_cc = use_cc and l < nlayers - 1
                ln_finish(st2, r2, h, h_bf, bia[:, :, 7:8],
                          bia[:, :, 8:9], defer=dts,
                          mid_hook=(lambda: cc_half(0)) if do_cc else None)

                if do_cc:
                    cc_half(1)
                    for o, bsl in dts:
                        with nc.allow_low_precision(reason="f32r LN output"):
                            nc.gpsimd.tensor_scalar(h[:, o, :], bsl,
                                                    bia[:, o, 7:8],
                                                    bia[:, o, 8:9], OP.mult, OP.add)
                else:
                    for o, bsl in dts:
                        with nc.allow_low_precision(reason="f32r LN output"):
                            nc.vector.tensor_scalar(h[:, o, :], bsl,
                                                    bia[:, o, 7:8],
                                                    bia[:, o, 8:9], OP.mult, OP.add)
                    if not use_cc and l < nlayers - 1:
                        # no-cc debug path: "sum" = 2*h so hrem = h (self-pair)
                        hg_out_prev = dram.tile([128, KD * TO], bf16, tag="hgout")
                        hdub = sb2.tile([128, KD, TO], bf16, tag="hsum", bufs=1,
                                        name="hdub")
                        nc.vector.tensor_scalar(hdub[:], h_bf[:], 2.0, None, OP.mult)
                        nc.sync.dma_start(hg_out_prev[:],
                                          hdub[:].rearrange("p a b -> p (a b)"))

            nc.sync.dma_start(out.rearrange("(ko kp) t -> kp ko t", kp=128),
                              h[:].bitcast(f32))

    nc.compile()
    return nc


def _selc():
    sel = np.zeros((16, KD * 128), np.float32)
    for t in range(KD):
        for m in range(128):
            sel[2 * t + m // 64, t * 128 + m] = 1.0
    return sel


def _pos_encoding(position, d):
    pos = np.arange(position)[:, None].astype(np.float32)
    i = np.arange(d)[None, :].astype(np.float32)
    angle = pos / np.power(10000.0, 2.0 * np.floor(i / 2.0) / np.float32(d))
    angle[:, 0::2] = np.sin(angle[:, 0::2])
    angle[:, 1::2] = np.cos(angle[:, 1::2])
    return angle.astype(np.float32)  # [position, d]


def _get_nc():
    if "nc" not in _cache:
        _cache["nc"] = build()
    return _cache["nc"]


def prepare_in_maps(inputs):
    """Host-side prep: pack weights/biases, shard tokens across 8 cores."""
    import ml_dtypes
    bf = ml_dtypes.bfloat16 if WBF else np.float32
    inp = {k: np.asarray(v, dtype=np.float32) for k, v in inputs.items()}
    pe = _pos_encoding(MAX_POS, D)[:S]
    x = inp["x"] + pe[None]

    def lhsT_pack(w):  # [L, D, D] -> [L, t2, kp, a, k, col]
        return np.ascontiguousarray(
            w.reshape(L, KD, 128, KD // 2, 2, 128).transpose(0, 3, 2, 4, 1, 5)
        ).astype(bf)

    common = {
        "WqT": lhsT_pack(inp["Wq"]),
        "WkT": lhsT_pack(inp["Wk"]),
        "WoT": lhsT_pack(inp["Wo"]),
        "WvN": np.ascontiguousarray(
            inp["Wv"].reshape(L, KD, 128, 4, 256).transpose(0, 3, 2, 1, 4)).astype(bf),
        "W1T": np.ascontiguousarray(
            inp["W1"].reshape(L, KD, 128, FT // 2, 2, 128)
            .transpose(0, 3, 2, 4, 1, 5)).astype(bf),
        "W2T": np.ascontiguousarray(
            inp["W2"].reshape(L, 2, FH, 128, KD, 128)
            .transpose(0, 1, 4, 3, 2, 5)).astype(bf),
    }
    pk = lambda a: np.ascontiguousarray(a.reshape(L, KD, 128).transpose(0, 2, 1))
    common["bias9"] = np.ascontiguousarray(np.stack(
        [pk(inp[k]) for k in ["bq", "bk", "bv", "bo", "b2", "g1", "be1", "g2", "be2"]],
        axis=-1))
    common["b1h"] = np.ascontiguousarray(
        inp["b1"].reshape(L, FT, 128).transpose(0, 2, 1)[..., None])
    common["cst"] = np.ones((128, 65), np.float32)
    common["cstb"] = np.ones((128, 64), ml_dtypes.bfloat16)
    common["crow"] = np.ones((65, 256), np.float32)
    common["selc"] = _selc()
    in_maps = []
    for c in range(NCORES):
        b, r = c // 2, c % 2
        m = dict(common)
        own = x[b, r * TO:(r + 1) * TO, :]
        oth = x[b, (1 - r) * TO:(2 - r) * TO, :]
        m["xTf"] = np.ascontiguousarray(own.T)
        m["xTb"] = np.ascontiguousarray(
            np.concatenate([own, oth], axis=0).T.astype(ml_dtypes.bfloat16))
        mk = (inp["mask"][b, 0, 0] * np.float32(-1e9)).astype(np.float32)
        mk4 = mk.reshape(4, 128)
        order = [2 * r, 2 * r + 1, 2 * (1 - r), 2 * (1 - r) + 1]
        m["msk"] = np.ascontiguousarray(mk4[order].T)
        in_maps.append(m)
    return in_maps


def kernel(**inputs):
    _, _, _, _, run_bass_kernel_spmd = _imports()
    nc = _get_nc()
    in_maps = prepare_in_maps(inputs)
    res = run_bass_kernel_spmd(nc, in_maps, core_ids=list(range(NCORES)))
    B = np.asarray(inputs["x"]).shape[0]
    out = np.stack([
        np.concatenate([res.results[2 * b]["out"].T,
                        res.results[2 * b + 1]["out"].T], axis=0)
        for b in range(B)])
    return out.astype(np.float32)
